# revision 30
# baseline (speedup 1.0000x reference)
"""Trainium2 Bass kernel for masked single-head attention.

Reference computation (per batch b):
    Q = q_hidden[b] @ Wq + bq            # [S, D]
    K = k_hidden[b] @ Wk + bk            # [S, D]
    V = v_hidden[b] @ Wv + bv            # [S, D]
    S_qk = (Q @ K.T) / sqrt(D)           # [S, S]
    S_qk = where(mask[b]==0, -1e9, S_qk)
    out[b] = softmax(S_qk, -1) @ V       # [S, D]

Sharding: data-parallel over batch, one batch per NeuronCore (B == 8
cores), no collectives.  bv folds into the output on the host (softmax
rows sum to 1).

Design notes (per core, S=2048, HID=1024, D=64).  The kernel is bound by
three near-equal resources: the TensorE matmul row stream (~130K rows at
~0.42ns/row sustained), the ScalarE(ACT) exp stream (S^2 columns,
~40us), and HBM delivery (~16.3MB at ~115GB/s per DMA queue, ~330GB/s
across the three issuing engines).  Every major choice below serves one
of those:

 - host ships transposed hiddens qT/kT/vT [HID, S] f16, the mask as
   48*(mT-1) fp8 {0,-48} (exact in e4m3), doubled weights [W|W] packed
   in one tensor, and fp8/f16 identity matrices.  Wq pre-scaled by
   1/sqrt(D).
 - projections: lhsT=[W|W] [128,128] writes both PSUM partition groups
   in one 512-row stream per chunk (the row-duplicated QT/KT layout the
   row-packed score matmuls need at zero extra PE rows); one DVE cast
   to f16 per chunk.
 - scores: unit (qc, p) = row-packed pair (kt 2p on partitions 0-63,
   2p+1 on 64-127), one PSUM bank per half, 6-bank rotation.  The two
   halves drain through different engines every unit to keep the drain
   off the PE critical path:
     half a: fp8 idm @ moffs accumulated on the PE, then one direct
       ACT exp PSUM->SBUF;
     half b: DVE tensor_add(st, moffs) PSUM->f16 SBUF, exp deferred
       (in-place on ACT) until after the scores/V-proj phase.
 - P tiles alias 1KB slots of qh/kh, whose projection reads end exactly
   when the P tiles are born: no pool close (a scoped-pool drain waits
   for entire DMA queues), per-slice WAR semaphores do the ordering.
   b-slots sit in kh column-chunks whose interleaved-kproj reads have
   already completed.
 - out^T[65, q] += [V|1].T @ P^T over k-tiles (row 64 = softmax
   denominator, ones column memset once); outT accumulators rotate in
   the same 6-bank PSUM tag.  norm: cast f16, PE-transpose, DVE
   reciprocal of the denominator column read straight from PSUM,
   tensor_scalar multiply, one 2D output DMA per q-chunk ([qc, p, t*D+d]
   layout, untangled on host).
 - DMA: 2D descriptors only (>=2KB contiguous runs; 3D APs fall back to
   slow swdge), striped round-robin over the sync/scalar/gpsimd queues
   in consumption order; scalar carries only front tensors so its exp
   stream is never stuck behind dma_start ring-space waits; mask tiles
   ride ahead of / between the v chunks; emission of DMAs is interleaved
   with compute stages because the tile scheduler merges semaphore waits
   up to everything emitted earlier (an all-DMAs-first prologue made the
   first matmul wait for ~32 transfers).
 - kproj chunks 1-3 and V-proj/V-finish are emitted between score
   groups so the PE has independent work while k/v rows land; a dummy
   activation at the top preloads the ACT exp table.
"""

import numpy as np
import ml_dtypes

import concourse.bass as bass
import concourse.tile as tile
from concourse import bacc
from concourse import mybir
from concourse.bass_utils import run_bass_kernel_spmd

B, S, HID, D = 8, 2048, 1024, 64
NCORES = 8
HCH = HID // 128          # 8 hidden chunks
KT_TILES = S // 128       # 16 k tiles
NQ = 512                  # q chunk width for the attention inner loop
QCH = S // NQ             # 4
NPAIR = KT_TILES // 2     # 8 k-tile pairs
MASK_C = 48.0             # mask offset (48 exactly representable in e4m3)

F32 = mybir.dt.float32
F16 = mybir.dt.float16
FP8 = mybir.dt.float8e4
F16_NP = np.float16
FP8_NP = ml_dtypes.float8_e4m3

LAST_EXEC_TIME_NS = None
_CACHED = {}


def _build_program(with_qk_bias=False, reps=1):
    nc = bacc.Bacc("TRN2", target_bir_lowering=False, debug=False,
                   num_swdge_queues=4)

    qT_d = nc.dram_tensor("qT", [HID, S], F16, kind="ExternalInput").ap()
    kT_d = nc.dram_tensor("kT", [HID, S], F16, kind="ExternalInput").ap()
    vT_d = nc.dram_tensor("vT", [HID, S], F16, kind="ExternalInput").ap()
    # 48*(mask.T - 1): 0 where visible, -48 where masked
    maskT_d = nc.dram_tensor("maskT", [S, S], FP8, kind="ExternalInput").ap()
    # all three doubled weights [W | W], packed partition-major so one 2D
    # DMA ships them: wall[p, (j*8+h)*128 + d] = W2_j[h*128+p, d]
    wall_d = nc.dram_tensor("wall", [128, 3 * HCH * 128], F16,
                            kind="ExternalInput").ap()
    idm_d = nc.dram_tensor("idm", [128, 128], FP8, kind="ExternalInput").ap()
    if with_qk_bias:
        bq_d = nc.dram_tensor("bq", [D], F32, kind="ExternalInput").ap()
        bk_d = nc.dram_tensor("bk", [D], F32, kind="ExternalInput").ap()
    idf_d = nc.dram_tensor("idf", [128, 128], F16, kind="ExternalInput").ap()
    # [qc, p, t*D+d] layout so the output DMA is a clean 2D descriptor;
    # host untangles with a reshape/transpose.
    out_d = nc.dram_tensor("out", [QCH, 128, (NQ // 128) * D], F32,
                           kind="ExternalOutput").ap()

    ExpF = mybir.ActivationFunctionType.Exp

    def _body(tc):
        with tc.tile_pool(name="const", bufs=1) as const:
            w_all = const.tile([128, 3, HCH, 2 * D], F16, name="w_all")
            w_q, w_k, w_v = (w_all[:, j] for j in range(3))
            idf16 = const.tile([128, 128], F16, name="idf16")
            idm = const.tile([128, 128], FP8, name="idm")
            if with_qk_bias:
                b_q = const.tile([128, 1], F32, name="b_q")
                b_k = const.tile([128, 1], F32, name="b_k")
                nc.sync.dma_start(b_q[0:D, :], bq_d.unsqueeze(1))
                nc.sync.dma_start(b_q[64:64 + D, :], bq_d.unsqueeze(1))
                nc.sync.dma_start(b_k[0:D, :], bk_d.unsqueeze(1))
                nc.sync.dma_start(b_k[64:64 + D, :], bk_d.unsqueeze(1))
            else:
                b_q = b_k = None

            masksb = const.tile([128, KT_TILES, S], FP8, name="masksb")
            vh = const.tile([128, HCH, S], F16, name="vh")
            QT = const.tile([128, S], F16, name="QT")
            KT = const.tile([128, S], F16, name="KT")
            VT = const.tile([128, S], F16, name="VT")
            Vt = const.tile([128, KT_TILES, D + 1], F16, name="Vt")

            qh = const.tile([128, HCH, S], F16, name="qh")
            kh = const.tile([128, HCH, S], F16, name="kh")

            # pt halves alias onto qh/kh 1KB slots: those reads end with
            # the q/k projections exactly when the P tiles are born, and
            # the per-slice WAR tracking orders them without any pool
            # drain.  Unit u: a-half in qh slot u, b-half in kh slot u.
            def qslot(t, u):
                return t[:, u // 4, (u % 4) * NQ:(u % 4 + 1) * NQ]

            # Each issuing engine owns one ~115 GB/s DMA queue (sync /
            # scalar hw queues, gpsimd swdge); aggregate ~330 GB/s.
            # Stripe every tensor round-robin across the queues in
            # consumption order: wq, q (chunk 0 first), k chunks with the
            # mask tiles trickling between, v rows last.
            engs = [nc.sync, nc.scalar, nc.gpsimd]
            rr_state = [0, 0]

            def issue(dst, src):
                # front tensors: all three queues
                engs[rr_state[0] % 3].dma_start(dst, src)
                rr_state[0] += 1

            def issue2(dst, src):
                # back-half (masks, v): sync/gpsimd only — the scalar
                # engine must reach its exp stream unobstructed, and
                # dma_start blocks on DMA-ring space.
                (nc.sync, nc.gpsimd)[rr_state[1] % 2].dma_start(dst, src)
                rr_state[1] += 1

            def issue_mask(kt):
                issue2(masksb[:, kt, :], maskT_d[kt * 128:(kt + 1) * 128, :])

            warm = const.tile([1, 16], F16, name="warm")
            nc.vector.memset(warm, 0.0)
            nc.scalar.activation(warm, warm, ExpF)
            nc.sync.dma_start(w_all[:, 0], wall_d[:, 0:1024]
                              .rearrange("p (o d) -> p o d", o=HCH))
            nc.scalar.dma_start(w_all[:, 1], wall_d[:, 1024:2048]
                                .rearrange("p (o d) -> p o d", o=HCH))
            nc.gpsimd.dma_start(w_all[:, 2], wall_d[:, 2048:3072]
                                .rearrange("p (o d) -> p o d", o=HCH))
            nc.gpsimd.dma_start(idm, idm_d)
            nc.gpsimd.dma_start(idf16, idf_d)
            for h in range(HCH):
                issue(qh[:, h, 0:NQ], qT_d[h * 128:(h + 1) * 128, 0:NQ])

            with tc.tile_pool(name="stp", bufs=2, space="PSUM") as stp, \
                 tc.tile_pool(name="nsb", bufs=2) as nsb:

                def proj(hid_t, w_t, b_t, dest, c, copy_eng):
                    cs = slice(c * NQ, (c + 1) * NQ)
                    prj = stp.tile([128, NQ], F32, name="prj", tag="prj",
                                   bufs=2)
                    for h in range(HCH):
                        nc.tensor.matmul(
                            prj, lhsT=w_t[:, h, :], rhs=hid_t[:, h, cs],
                            start=(h == 0), stop=(h == HCH - 1))
                    copy_eng.tensor_copy(dest[:, cs], prj)
                    if b_t is not None:
                        copy_eng.tensor_scalar_add(dest[:, cs], dest[:, cs],
                                                   b_t)

                # ---- staged emission, DMA issues interleaved so the
                # scheduler's merged waits stay tight ----
                proj(qh, w_q, b_q, QT, 0, nc.vector)
                for h in range(HCH):
                    issue(qh[:, h, NQ:S], qT_d[h * 128:(h + 1) * 128, NQ:S])
                for c in range(1, QCH):
                    proj(qh, w_q, b_q, QT, c, nc.vector)
                for h in range(HCH):
                    issue(kh[:, h, 0:NQ], kT_d[h * 128:(h + 1) * 128, 0:NQ])
                for h in range(HCH):
                    issue(kh[:, h, NQ:S], kT_d[h * 128:(h + 1) * 128, NQ:S])
                proj(kh, w_k, b_k, KT, 0, nc.vector)
                # masks and v interleaved by first-need time on the two
                # back-half queues; v as [128, 1024] h-halves.
                vitems = [(h, c) for c in (0, 1) for h in range(HCH)] + \
                         [(h, c) for c in (2, 3) for h in range(HCH)]

                def issue_v(n):
                    for h, c in vitems[:n]:
                        issue2(vh[:, h, c * NQ:(c + 1) * NQ],
                               vT_d[h * 128:(h + 1) * 128,
                                    c * NQ:(c + 1) * NQ])
                    del vitems[:n]

                for kt in range(6):
                    issue_mask(kt)
                issue_v(4)
                for kt in range(6, 10):
                    issue_mask(kt)
                issue_v(6)
                for kt in range(10, 14):
                    issue_mask(kt)
                issue_v(4)
                issue_mask(14)
                issue_mask(15)
                issue_v(len(vitems))

                if True:

                    def sc_unit(qc, p):
                        # row-packed score pair (2p, 2p+1), one PSUM bank
                        # per k-tile so the drain pipeline runs deep.
                        # Half a: mask offsets accumulated on the PE (fp8
                        # idm @ moffs), drained by a direct ACT exp.
                        # Half b: drained by a DVE add of the mask
                        # offsets; its exp runs in-place, deferred.
                        u = 4 * p + qc
                        q0 = qc * NQ
                        qsl = slice(q0, q0 + NQ)
                        kta, ktb = 2 * p, 2 * p + 1
                        sa = slice(kta * 128, kta * 128 + 128)
                        sb = slice(ktb * 128, ktb * 128 + 128)
                        sta = stp.tile([128, NQ], F32, name="sta", tag="st",
                                       bufs=6)
                        nc.tensor.matmul(
                            sta, lhsT=KT[0:D, sa], rhs=QT[0:D, qsl],
                            start=True, stop=False)
                        nc.tensor.matmul(
                            sta, lhsT=idm, rhs=masksb[:, kta, qsl],
                            start=False, stop=True)
                        pta = qslot(qh, u)
                        nc.scalar.activation(pta, sta, ExpF)
                        stb = stp.tile([128, NQ], F32, name="stb", tag="st",
                                       bufs=6)
                        nc.tensor.matmul(
                            stb, lhsT=KT[64:64 + D, sb],
                            rhs=QT[64:64 + D, qsl], start=True, stop=True)
                        # b-slot lives in a kh column chunk (p//2) whose
                        # kproj reads completed before this group
                        pre = kh[:, qc + 4 * (p % 2),
                                 (p // 2) * NQ:(p // 2 + 1) * NQ]
                        nc.vector.tensor_add(pre, stb, masksb[:, ktb, qsl])
                        return pta, pre

                    def v_fin(kt):
                        vtr = stp.tile([128, D], F16, name="vtr", tag="prj",
                                       bufs=2)
                        nc.tensor.transpose(
                            vtr, VT[0:D, kt * 128:(kt + 1) * 128],
                            idf16[0:D, 0:D])
                        nc.vector.tensor_copy(Vt[:, kt, :D], vtr)

                    def av(outT, p, pta, ptb):
                        nc.tensor.matmul(
                            outT, lhsT=Vt[:, 2 * p, :], rhs=pta,
                            start=(p == 0), stop=False)
                        nc.tensor.matmul(
                            outT, lhsT=Vt[:, 2 * p + 1, :], rhs=ptb,
                            start=False, stop=(p == NPAIR - 1))

                    def norm(qc, outT):
                        outT_sb = nsb.tile([D + 1, NQ], F16, name="outT_sb",
                                           tag="outT_sb")
                        nc.vector.tensor_copy(outT_sb, outT)
                        o_big = nsb.tile([128, (NQ // 128) * D], F32,
                                         name="o_big", tag="o_big")
                        for i in range(NQ // 128):
                            tr = stp.tile([128, D + 1], F16, name="tr",
                                          tag="prj", bufs=2)
                            nc.tensor.transpose(
                                tr, outT_sb[:, i * 128:(i + 1) * 128],
                                idf16[:D + 1, :D + 1])
                            rcp = nsb.tile([128, 1], F32, name="rcp",
                                           tag="rcp")
                            nc.vector.reciprocal(rcp, tr[:, D:D + 1])
                            nc.vector.tensor_scalar_mul(
                                o_big[:, i * D:(i + 1) * D], tr[:, :D], rcp)
                        nc.sync.dma_start(out_d[qc], o_big)

                    # ones column of Vt written once
                    nc.gpsimd.memset(Vt[:, :, D:D + 1], 1.0)

                    # k-pair-major unit order: mask tile 2p is first
                    # needed ~2.6us * p into the phase, so mask DMAs can
                    # trickle in behind the q/k rows.
                    pts = {}
                    pres = {}
                    for p in range(NPAIR):
                        for qc in range(QCH):
                            pts[(qc, p)], pres[(qc, p)] = sc_unit(qc, p)
                        if p % 2 == 1 and p < 7:
                            # k chunk (p+3)//2... next score pairs need
                            # KT cols [(p+1)*256...]: chunk (p+1)//2
                            proj(kh, w_k, b_k, KT, (p + 1) // 2, nc.vector)
                        if p >= 4:
                            c = p - 4
                            proj(vh, w_v, None, VT, c, nc.vector)
                            for kt in range(4 * c, 4 * c + 4):
                                v_fin(kt)
                    # deferred in-place exps for the b-halves (ACT, off
                    # the PE critical path), fused over adjacent columns
                    for qc in range(QCH):
                        for h_off, c0 in ((0, 0), (4, 0), (0, 2), (4, 2)):
                            s = kh[:, qc + h_off, c0 * NQ:(c0 + 2) * NQ]
                            nc.scalar.activation(s, s, ExpF)
                    for qc in range(QCH):
                        outT = stp.tile([D + 1, NQ], F32, name="outT",
                                        tag="st", bufs=6)
                        for p in range(NPAIR):
                            av(outT, p, pts[(qc, p)], pres[(qc, p)])
                        norm(qc, outT)

    with tile.TileContext(nc) as tc:
        if reps > 1:
            with tc.For_i(0, reps, 1):
                _body(tc)
        else:
            _body(tc)

    nc.compile()
    return nc


def _prep_inputs(q_hidden_inputs, k_hidden_inputs, v_hidden_inputs, mask,
                 Wq, bq, Wk, bk, Wv, bv):
    scale = np.float32(1.0 / np.sqrt(np.float32(D)))
    wq = (np.asarray(Wq, np.float32) * scale).astype(F16_NP)
    wk = np.asarray(Wk, np.float32).astype(F16_NP)
    wv = np.asarray(Wv, np.float32).astype(F16_NP)
    wq2 = np.concatenate([wq, wq], axis=1)
    wk2 = np.concatenate([wk, wk], axis=1)
    wv2 = np.concatenate([wv, wv], axis=1)
    # wall[p, (j*8+h)*128 + d] = W2_j[h*128+p, d]
    wall = np.ascontiguousarray(
        np.stack([wq2, wk2, wv2])               # [3, HID, 2D]
        .reshape(3, HCH, 128, 2 * D)            # [3, h, p, d]
        .transpose(2, 0, 1, 3)                  # [p, 3, h, d]
        .reshape(128, 3 * HCH * 2 * D))
    bqs = (np.asarray(bq, np.float32) * scale)
    bks = np.asarray(bk, np.float32)
    with_qk_bias = bool(np.any(bqs != 0) or np.any(bks != 0))
    idf = np.eye(128, dtype=np.float32).astype(F16_NP)
    idm = np.eye(128, dtype=np.float32).astype(FP8_NP)

    q = np.asarray(q_hidden_inputs, np.float32)
    k = np.asarray(k_hidden_inputs, np.float32)
    v = np.asarray(v_hidden_inputs, np.float32)
    m = np.asarray(mask)

    in_maps = []
    for b in range(B):
        im = {
            "qT": np.ascontiguousarray(q[b].T).astype(F16_NP),
            "kT": np.ascontiguousarray(k[b].T).astype(F16_NP),
            "vT": np.ascontiguousarray(v[b].T).astype(F16_NP),
            "maskT": ((np.ascontiguousarray(m[b].T) - np.int32(1)) *
                      np.float32(MASK_C)).astype(FP8_NP),
            "wall": wall, "idm": idm,
            "idf": idf,
        }
        if with_qk_bias:
            im["bq"] = bqs
            im["bk"] = bks
        in_maps.append(im)
    return in_maps, with_qk_bias


def kernel(q_hidden_inputs, k_hidden_inputs, v_hidden_inputs, mask,
           Wq, bq, Wk, bk, Wv, bv, trace=False):
    global LAST_EXEC_TIME_NS
    in_maps, with_qk_bias = _prep_inputs(
        q_hidden_inputs, k_hidden_inputs, v_hidden_inputs,
        mask, Wq, bq, Wk, bk, Wv, bv)
    key = ("nc", with_qk_bias)
    if key not in _CACHED:
        _CACHED[key] = _build_program(with_qk_bias)
    nc = _CACHED[key]

    res = run_bass_kernel_spmd(nc, in_maps, list(range(NCORES)), trace=trace)
    LAST_EXEC_TIME_NS = res.exec_time_ns
    # out_d is [qc, p, t*D+d] with q = qc*512 + t*128 + p
    out = np.stack(
        [res.results[b]["out"].reshape(QCH, 128, NQ // 128, D)
         .transpose(0, 2, 1, 3).reshape(S, D) for b in range(B)], axis=0)
    # bv folds into the output exactly: softmax rows sum to 1, so
    # attn @ (V + 1 bv^T) = attn @ V + bv.
    out = out + np.asarray(bv, np.float32)[None, None, :]
    return out
